# revision 4
# baseline (speedup 1.0000x reference)
"""DetectHead (three 1x1-conv heads fused) on 8 Trainium2 NeuronCores.

Math: out[b,h,w,:] = concat(cls, box, dir) = W_all @ x[b,:,h,w] + bias_all
with W_all = concat(cls_w, box_w, dir_w) in R^{72x1024}.

Sharding: 8 shards = (batch, H-half). Each core processes a contiguous
(1024, 100*176=17600) slice of x and produces (17600, 72) of the
channels-last output. The kernel is HBM-read-bound (~72 MB fp32 per core),
so everything else (matmuls, transposes, bias add, output writes) is
hidden under the input DMA stream.

Per 512-pixel tile on each core:
  - one 2.1 MB DMA loads x[1024c, 512pix] as SBUF [128p, 8k, 512]
  - 8 accumulating float32r matmuls (stationary W chunk [128,72],
    moving x chunk [128,512]) -> PSUM [72, 512]
  - DVE copies PSUM -> SBUF, then 4 PE transposes ([72,128] -> [128,72])
  - DVE adds broadcast bias, one DMA writes the contiguous
    (512, 72) pixel-major block to DRAM.
"""

import numpy as np
from contextlib import ExitStack

import concourse.bass as bass
import concourse.tile as tile
from concourse import bacc, mybir
from concourse.bass_utils import run_bass_kernel_spmd

B, C, H, W = 4, 1024, 200, 176
HH = H // 2            # 100 rows of H per shard
PIX = HH * W           # 17600 pixels per shard
NCORES = 8
KCH = C // 128         # 8 channel chunks
O = 72                 # 18 cls + 42 box + 12 dir output channels
TILE_N = 512
FULL_TILES = PIX // TILE_N          # 34
TAIL = PIX - FULL_TILES * TILE_N    # 192

F32 = mybir.dt.float32
F32R = mybir.dt.float32r

_compiled = {}


def _build_program():
    nc = bacc.Bacc(
        "TRN2", target_bir_lowering=False, debug=False, num_devices=NCORES
    )
    xs = nc.dram_tensor("xs", [C, PIX], F32R, kind="ExternalInput").ap()
    wt = nc.dram_tensor("wt", [128, KCH * O], F32R, kind="ExternalInput").ap()
    biasbc = nc.dram_tensor("biasbc", [128, 4 * O], F32, kind="ExternalInput").ap()
    ident = nc.dram_tensor("ident", [O, O], F32, kind="ExternalInput").ap()
    out = nc.dram_tensor("out", [PIX, O], F32, kind="ExternalOutput").ap()

    # [c, pix] viewed as [p, k, pix] with c = k*128 + p
    xs_v = xs.rearrange("(k p) n -> p k n", k=KCH)

    with tile.TileContext(nc) as tc, ExitStack() as ctx:
        cpool = ctx.enter_context(tc.tile_pool(name="consts", bufs=1))
        xpool = ctx.enter_context(tc.tile_pool(name="xin", bufs=3))
        spool = ctx.enter_context(tc.tile_pool(name="stage", bufs=3))
        opool = ctx.enter_context(tc.tile_pool(name="outsb", bufs=3))
        mpool = ctx.enter_context(tc.tile_pool(name="pmm", bufs=2, space="PSUM"))
        tpool = ctx.enter_context(tc.tile_pool(name="ptr", bufs=2, space="PSUM"))

        w_sb = cpool.tile([128, KCH * O], F32R)
        nc.sync.dma_start(out=w_sb[:, :], in_=wt[:, :])
        bias_sb = cpool.tile([128, 4 * O], F32)
        nc.sync.dma_start(out=bias_sb[:, :], in_=biasbc[:, :])
        id_sb = cpool.tile([O, O], F32)
        nc.sync.dma_start(out=id_sb[:, :], in_=ident[:, :])

        def do_tile(pix0, n):
            # number of 128-pixel transpose chunks in this tile
            njs = [128] * (n // 128)
            if n % 128:
                njs.append(n % 128)
            nj = len(njs)

            xbuf = xpool.tile([128, KCH, n], F32R, tag="xbuf")
            nc.sync.dma_start(out=xbuf[:, :, :], in_=xs_v[:, :, pix0 : pix0 + n])

            pmm = mpool.tile([O, n], F32, tag="pmm")
            for k in range(KCH):
                nc.tensor.matmul(
                    pmm[:, :],
                    w_sb[:, k * O : (k + 1) * O],
                    xbuf[:, k, :],
                    start=(k == 0),
                    stop=(k == KCH - 1),
                )

            s1 = spool.tile([O, n], F32, tag="s1")
            nc.vector.tensor_copy(s1[:, :], pmm[:, :])

            pt = tpool.tile([128, nj * O], F32, tag="pt")
            for j, pj in enumerate(njs):
                nc.tensor.transpose(
                    pt[:pj, j * O : (j + 1) * O],
                    s1[:, j * 128 : j * 128 + pj],
                    id_sb[:, :],
                )

            ot = opool.tile([128, nj * O], F32, tag="ot")
            if n % 128 == 0:
                nc.vector.tensor_add(ot[:, :], pt[:, :], bias_sb[:, : nj * O])
                nc.sync.dma_start(
                    out=out[pix0 : pix0 + n, :].rearrange("(j p) o -> p j o", p=128),
                    in_=ot[:, :].rearrange("p (j o) -> p j o", j=nj),
                )
            else:
                for j, pj in enumerate(njs):
                    nc.vector.tensor_add(
                        ot[:pj, j * O : (j + 1) * O],
                        pt[:pj, j * O : (j + 1) * O],
                        bias_sb[:pj, j * O : (j + 1) * O],
                    )
                    nc.sync.dma_start(
                        out=out[pix0 + j * 128 : pix0 + j * 128 + pj, :],
                        in_=ot[:pj, j * O : (j + 1) * O],
                    )

        for t in range(FULL_TILES):
            do_tile(t * TILE_N, TILE_N)
        if TAIL:
            do_tile(FULL_TILES * TILE_N, TAIL)

    nc.compile()
    return nc


def _get_program():
    if "nc" not in _compiled:
        _compiled["nc"] = _build_program()
    return _compiled["nc"]


def _make_in_maps(x, cls_w, cls_b, box_w, box_b, dir_w, dir_b):
    w_all = np.concatenate(
        [np.asarray(cls_w), np.asarray(box_w), np.asarray(dir_w)], axis=0
    ).astype(np.float32)  # (72, 1024)
    bias_all = np.concatenate(
        [np.asarray(cls_b), np.asarray(box_b), np.asarray(dir_b)]
    ).astype(np.float32)  # (72,)

    # wt[p, k*O + o] = w_all[o, k*128 + p]
    wt = np.ascontiguousarray(
        w_all.T.reshape(KCH, 128, O).transpose(1, 0, 2).reshape(128, KCH * O)
    )
    biasbc = np.ascontiguousarray(np.tile(bias_all, (128, 4)))
    ident = np.eye(O, dtype=np.float32)

    x = np.asarray(x)
    in_maps = []
    for i in range(NCORES):
        b, half = divmod(i, 2)
        xs = np.ascontiguousarray(
            x[b, :, half * HH : (half + 1) * HH, :]
        ).reshape(C, PIX)
        in_maps.append({"xs": xs, "wt": wt, "biasbc": biasbc, "ident": ident})
    return in_maps


def _gather(results):
    out = np.empty((B, H, W, O), dtype=np.float32)
    for i in range(NCORES):
        b, half = divmod(i, 2)
        out[b, half * HH : (half + 1) * HH] = results[i]["out"].reshape(HH, W, O)
    return out


def kernel(x, cls_w, cls_b, box_w, box_b, dir_w, dir_b):
    nc = _get_program()
    in_maps = _make_in_maps(x, cls_w, cls_b, box_w, box_b, dir_w, dir_b)
    res = run_bass_kernel_spmd(nc, in_maps, list(range(NCORES)))
    return _gather(res.results)


def kernel_profiled(x, cls_w, cls_b, box_w, box_b, dir_w, dir_b, **trace_kwargs):
    """Like kernel() but requests an NTFF trace; returns (output, BassKernelResults)."""
    nc = _get_program()
    in_maps = _make_in_maps(x, cls_w, cls_b, box_w, box_b, dir_w, dir_b)
    res = run_bass_kernel_spmd(
        nc, in_maps, list(range(NCORES)), trace=True, **trace_kwargs
    )
    return _gather(res.results), res


# revision 6
# speedup vs baseline: 20.3537x; 20.3537x over previous
"""DetectHead (three 1x1-conv heads fused) on 8 Trainium2 NeuronCores.

Math: out[b,h,w,:] = concat(cls, box, dir) = W_all @ x[b,:,h,w] + bias_all
with W_all = concat(cls_w, box_w, dir_w) in R^{72x1024}.

Sharding: 8 shards = (batch, H-half). Each core processes a contiguous
(1024, 100*176=17600) slice of x and produces (17600, 72) of the
channels-last output. The kernel is HBM-read-bound (~72 MB fp32 per core),
so everything else (matmuls, transposes, bias add, output writes) is
hidden under the input DMA stream.

Per 512-pixel tile on each core:
  - one 2.1 MB DMA loads x[1024c, 512pix] as SBUF [128p, 8k, 512]
  - 8 accumulating float32r matmuls (stationary W chunk [128,72],
    moving x chunk [128,512]) -> PSUM [72, 512]
  - DVE copies PSUM -> SBUF, then 4 PE transposes ([72,128] -> [128,72])
  - DVE adds broadcast bias, one DMA writes the contiguous
    (512, 72) pixel-major block to DRAM.
"""

import numpy as np
from contextlib import ExitStack

import concourse.bass as bass
import concourse.tile as tile
from concourse import bacc, mybir
from concourse.bass_utils import run_bass_kernel_spmd

B, C, H, W = 4, 1024, 200, 176
HH = H // 2            # 100 rows of H per shard
PIX = HH * W           # 17600 pixels per shard
NCORES = 8
KCH = C // 128         # 8 channel chunks
O = 72                 # 18 cls + 42 box + 12 dir output channels
TILE_N = 512
FULL_TILES = PIX // TILE_N          # 34
TAIL = PIX - FULL_TILES * TILE_N    # 192

F32 = mybir.dt.float32
F32R = mybir.dt.float32r

_compiled = {}


def _build_program(repeat=1):
    nc = bacc.Bacc(
        "TRN2", target_bir_lowering=False, debug=False, num_devices=NCORES
    )
    xs = nc.dram_tensor("xs", [C, PIX], F32R, kind="ExternalInput").ap()
    wt = nc.dram_tensor("wt", [128, KCH * O], F32R, kind="ExternalInput").ap()
    biasbc = nc.dram_tensor("biasbc", [128, 4 * O], F32, kind="ExternalInput").ap()
    ident = nc.dram_tensor("ident", [O, O], F32, kind="ExternalInput").ap()
    out = nc.dram_tensor("out", [PIX, O], F32, kind="ExternalOutput").ap()

    # [c, pix] viewed as [p, k, pix] with c = k*128 + p
    xs_v = xs.rearrange("(k p) n -> p k n", k=KCH)

    with tile.TileContext(nc) as tc, ExitStack() as ctx:
        cpool = ctx.enter_context(tc.tile_pool(name="consts", bufs=1))
        xpool = ctx.enter_context(tc.tile_pool(name="xin", bufs=4))
        spool = ctx.enter_context(tc.tile_pool(name="stage", bufs=3))
        opool = ctx.enter_context(tc.tile_pool(name="outsb", bufs=3))
        mpool = ctx.enter_context(tc.tile_pool(name="pmm", bufs=2, space="PSUM"))
        tpool = ctx.enter_context(tc.tile_pool(name="ptr", bufs=2, space="PSUM"))

        w_sb = cpool.tile([128, KCH * O], F32R)
        nc.sync.dma_start(out=w_sb[:, :], in_=wt[:, :])
        bias_sb = cpool.tile([128, 4 * O], F32)
        nc.sync.dma_start(out=bias_sb[:, :], in_=biasbc[:, :])
        id_sb = cpool.tile([O, O], F32)
        nc.sync.dma_start(out=id_sb[:, :], in_=ident[:, :])

        def do_mm_tile(xbuf, off, pix0, n):
            # one matmul pipeline over n<=512 pixels at offset `off` in xbuf
            njs = [128] * (n // 128)
            if n % 128:
                njs.append(n % 128)
            nj = len(njs)

            pmm = mpool.tile([O, n], F32, tag="pmm")
            for k in range(KCH):
                nc.tensor.matmul(
                    pmm[:, :],
                    w_sb[:, k * O : (k + 1) * O],
                    xbuf[:, k, off : off + n],
                    start=(k == 0),
                    stop=(k == KCH - 1),
                )

            s1 = spool.tile([O, n], F32, tag="s1")
            nc.vector.tensor_copy(s1[:, :], pmm[:, :])

            pt = tpool.tile([128, nj * O], F32, tag="pt")
            for j, pj in enumerate(njs):
                nc.tensor.transpose(
                    pt[:pj, j * O : (j + 1) * O],
                    s1[:, j * 128 : j * 128 + pj],
                    id_sb[:, :],
                )

            ot = opool.tile([128, nj * O], F32, tag="ot")
            if n % 128 == 0:
                nc.vector.tensor_add(ot[:, :], pt[:, :], bias_sb[:, : nj * O])
                nc.scalar.dma_start(
                    out=out[pix0 : pix0 + n, :].rearrange("(j p) o -> p j o", p=128),
                    in_=ot[:, :].rearrange("p (j o) -> p j o", j=nj),
                )
            else:
                for j, pj in enumerate(njs):
                    nc.vector.tensor_add(
                        ot[:pj, j * O : (j + 1) * O],
                        pt[:pj, j * O : (j + 1) * O],
                        bias_sb[:pj, j * O : (j + 1) * O],
                    )
                    nc.scalar.dma_start(
                        out=out[pix0 + j * 128 : pix0 + j * 128 + pj, :],
                        in_=ot[:pj, j * O : (j + 1) * O],
                    )

        def do_group(pix0, n):
            # one input DMA covering n pixels (up to GROUP), then MM tiles of 512
            xbuf = xpool.tile([128, KCH, n], F32R, tag="xbuf")
            nc.sync.dma_start(out=xbuf[:, :, :], in_=xs_v[:, :, pix0 : pix0 + n])
            off = 0
            while off < n:
                m = min(TILE_N, n - off)
                do_mm_tile(xbuf, off, pix0 + off, m)
                off += m

        GROUP = 2 * TILE_N  # 1024 pixels -> 4.2 MB per input DMA
        for _rep in range(repeat):
            g0 = 0
            while g0 < PIX:
                gn = min(GROUP, PIX - g0)
                do_group(g0, gn)
                g0 += gn

    nc.compile()
    return nc


def _get_program(repeat=1):
    if repeat not in _compiled:
        _compiled[repeat] = _build_program(repeat)
    return _compiled[repeat]


def _make_in_maps(x, cls_w, cls_b, box_w, box_b, dir_w, dir_b):
    w_all = np.concatenate(
        [np.asarray(cls_w), np.asarray(box_w), np.asarray(dir_w)], axis=0
    ).astype(np.float32)  # (72, 1024)
    bias_all = np.concatenate(
        [np.asarray(cls_b), np.asarray(box_b), np.asarray(dir_b)]
    ).astype(np.float32)  # (72,)

    # wt[p, k*O + o] = w_all[o, k*128 + p]
    wt = np.ascontiguousarray(
        w_all.T.reshape(KCH, 128, O).transpose(1, 0, 2).reshape(128, KCH * O)
    )
    biasbc = np.ascontiguousarray(np.tile(bias_all, (128, 4)))
    ident = np.eye(O, dtype=np.float32)

    x = np.asarray(x)
    in_maps = []
    for i in range(NCORES):
        b, half = divmod(i, 2)
        xs = np.ascontiguousarray(
            x[b, :, half * HH : (half + 1) * HH, :]
        ).reshape(C, PIX)
        in_maps.append({"xs": xs, "wt": wt, "biasbc": biasbc, "ident": ident})
    return in_maps


def _gather(results):
    out = np.empty((B, H, W, O), dtype=np.float32)
    for i in range(NCORES):
        b, half = divmod(i, 2)
        out[b, half * HH : (half + 1) * HH] = results[i]["out"].reshape(HH, W, O)
    return out


def kernel(x, cls_w, cls_b, box_w, box_b, dir_w, dir_b):
    nc = _get_program()
    in_maps = _make_in_maps(x, cls_w, cls_b, box_w, box_b, dir_w, dir_b)
    res = run_bass_kernel_spmd(nc, in_maps, list(range(NCORES)))
    return _gather(res.results)


def kernel_profiled(x, cls_w, cls_b, box_w, box_b, dir_w, dir_b, **trace_kwargs):
    """Like kernel() but requests an NTFF trace; returns (output, BassKernelResults)."""
    nc = _get_program()
    in_maps = _make_in_maps(x, cls_w, cls_b, box_w, box_b, dir_w, dir_b)
    res = run_bass_kernel_spmd(
        nc, in_maps, list(range(NCORES)), trace=True, **trace_kwargs
    )
    return _gather(res.results), res
